# revision 14
# baseline (speedup 1.0000x reference)
"""Trainium2 Bass kernel for nn_ConexaoRegional.

Reference computation:
    out[b, n, d, s] = sum_r xd[b, n, r] * wd[n, d, s, r]
where
    xd[b, (i,j), r] = x[b, 0, 4i+r, 4j+r]     (patch diagonal)
    wd[n, d, s, r]  = pesos[n, d, s, r, r]    (weight diagonal)

Shapes: x [64,1,128,128] f32, pesos [1024,16,32,4,4] f32,
        out [64,1024,16,32] f32 (128 MiB -> memory-bound).

Strategy: shard the region axis (n) across 8 cores (128 regions each,
contiguous patch-row blocks). Host packs, per core and per pair of
regions (2p, 2p+1), a block-diagonal stationary operand whose rows are
(term, c, r) and cols (c*64+b), and a moving operand [K, 512] with the
matching wd rows. fp32 matmul runs ~8x slower than bf16 on TRN2, so
operands are split hi/lo in bf16 and the four product terms
(hh, hl, lh, ll) are folded into the CONTRACTION dim: K = 32 rows =
[xh;xl;xh;xl] against [wh;wh;wl;wl]. bf16 products are exact in the
fp32 PSUM accumulation, so the result carries only the double-bf16
representation error (~8e-6 relative). One 213 ns matmul per pair ->
PSUM [128, 512] = 2 regions x 64 batch x 512 (d,s). PSUM->SBUF copies
alternate Vector/Scalar engines; stores go out as one fully-contiguous
2 MiB DMA per 8-pair chunk (the DRAM buffer is written in engine order
[chunk, c, b, pair, ds] and un-permuted on the host). All FLOPs (the
einsum contraction) happen on device.
"""

import numpy as np

B = 64
R = 4
GH = GW = 32
N = GH * GW            # 1024 regions
D, S = 16, 32
DS = D * S             # 512
NCORES = 8
NPC = N // NCORES      # 128 regions per core
PAIRS = NPC // 2       # 64 pair-matmuls per core
GCH = 4                # pairs per output store chunk
NCHUNK = PAIRS // GCH  # 16 chunks -> 1 MiB contiguous per store DMA
CHUNK_ELEMS = 2 * B * GCH * DS  # 262144 f32 per chunk

_NC_CACHE = {}


def _build_bass():
    if "nc" in _NC_CACHE:
        return _NC_CACHE["nc"]
    from contextlib import ExitStack

    import concourse.bacc as bacc
    import concourse.mybir as mybir
    import concourse.tile as tile

    f32 = mybir.dt.float32
    bf16 = mybir.dt.bfloat16
    nc = bacc.Bacc()  # Bacc (not raw Bass): its compile passes split multi-sem
    # waits and move matmul waits to ldweights, which TRN2 codegen requires.

    # K = 32 rows: 4 hi/lo term blocks of 8 rows (c*4+r).
    xbd = nc.declare_dram_parameter("xbd", [32, PAIRS * 128], bf16, isOutput=False)
    wt = nc.declare_dram_parameter("wt", [32, PAIRS * DS], bf16, isOutput=False)
    out = nc.declare_dram_parameter("out", [NCHUNK, CHUNK_ELEMS], f32, isOutput=True)

    with ExitStack() as ctx:
        tc = ctx.enter_context(tile.TileContext(nc))
        const = ctx.enter_context(tc.tile_pool(name="const", bufs=1))
        wpool = ctx.enter_context(tc.tile_pool(name="wtp", bufs=3))
        pspool = ctx.enter_context(tc.tile_pool(name="ps", bufs=8, space="PSUM"))
        opool = ctx.enter_context(tc.tile_pool(name="ostage", bufs=3))

        # Issue chunk 0's weight load and the first slice of the stationary
        # operand first so the pipeline starts streaming ASAP; the bulk of
        # the stationary tensor arrives while chunk 0 computes.
        wsb0 = wpool.tile([32, GCH * DS], bf16, name="wsb0", tag="wsb")
        nc.sync.dma_start(wsb0[:], wt[:, 0:GCH * DS])
        xsb = const.tile([32, PAIRS * 128], bf16)
        head = 2 * GCH * 128
        nc.sync.dma_start(xsb[:, :head], xbd[:, :head])
        nc.sync.dma_start(xsb[:, head:], xbd[:, head:])

        for g in range(NCHUNK):
            if g == 0:
                wsb = wsb0
            else:
                wsb = wpool.tile([32, GCH * DS], bf16, name=f"wsb{g}", tag="wsb")
                nc.sync.dma_start(wsb[:], wt[:, g * GCH * DS:(g + 1) * GCH * DS])
            ostage = opool.tile([128, GCH * DS], f32)
            for j in range(GCH):
                p = g * GCH + j
                ps = pspool.tile([128, DS], f32)
                nc.tensor.matmul(
                    ps[:],
                    lhsT=xsb[:, p * 128:(p + 1) * 128],
                    rhs=wsb[:, j * DS:(j + 1) * DS],
                    start=True,
                    stop=True,
                )
                dst = ostage[:, j * DS:(j + 1) * DS]
                if j % 2 == 0:
                    nc.vector.tensor_copy(dst, ps[:])
                else:
                    nc.scalar.copy(dst, ps[:])
            # ostage rows (c*64+b), free (j, ds) -> out[g] is written in
            # exactly that iteration order, so the store is contiguous.
            nc.sync.dma_start(out[g], ostage[:])

    nc.compile()  # Bacc passes: reg alloc, wait splitting, ldweights fixup
    _NC_CACHE["nc"] = nc
    return nc


def _pack_inputs(x, pesos):
    import ml_dtypes

    bf16 = ml_dtypes.bfloat16
    x = np.ascontiguousarray(np.asarray(x), dtype=np.float32)
    pesos = np.ascontiguousarray(np.asarray(pesos), dtype=np.float32)
    # xd[b, i, j, r] = x[b, 0, 4i+r, 4j+r]
    xp = x.reshape(B, GH, R, GW, R)
    xd = np.einsum("birjr->bijr", xp).reshape(B, N, R)
    # wd[n, ds, r] = pesos[n, d, s, r, r]
    wd = pesos.reshape(N, DS, R * R)[:, :, :: R + 1]  # [N, 512, 4]

    def hilo(a):
        hi = a.astype(bf16)
        lo = (a - hi.astype(np.float32)).astype(bf16)
        return hi, lo

    in_maps = []
    for k in range(NCORES):
        n0 = k * NPC
        xdk = xd[:, n0:n0 + NPC, :]   # [B, 128, 4]
        wdk = wd[n0:n0 + NPC]         # [128, 512, 4]
        # wt[c*4+r, p*512+ds] = wdk[2p+c, ds, r]
        wtk = np.ascontiguousarray(
            wdk.reshape(PAIRS, 2, DS, R).transpose(1, 3, 0, 2)
        ).reshape(8, PAIRS * DS)
        # xbd[c*4+r, p*128 + c*64 + b] = xdk[b, 2p+c, r] (block-diagonal)
        A = xdk.reshape(B, PAIRS, 2, R).transpose(2, 3, 1, 0)  # [c, r, p, b]
        L = np.zeros((2, R, PAIRS, 2, B), dtype=np.float32)
        L[0, :, :, 0, :] = A[0]
        L[1, :, :, 1, :] = A[1]
        xbdk = L.reshape(8, PAIRS * 128)
        xh, xl = hilo(xbdk)
        wh, wl = hilo(wtk)
        in_maps.append(
            {
                "xbd": np.ascontiguousarray(np.concatenate([xh, xl, xh, xl], axis=0)),
                "wt": np.ascontiguousarray(np.concatenate([wh, wh, wl, wl], axis=0)),
            }
        )
    return in_maps


TRACE = {"on": False, "last": None}


def kernel(x, pesos):
    from concourse.bass_utils import run_bass_kernel_spmd

    in_maps = _pack_inputs(x, pesos)
    nc = _build_bass()
    res = run_bass_kernel_spmd(
        nc, in_maps, core_ids=list(range(NCORES)), trace=TRACE["on"]
    )
    TRACE["last"] = res
    outs = []
    for k in range(NCORES):
        # res[g, c, b, j, ds] = out[b, GCH*2*g + 2j + c, ds]
        r = res.results[k]["out"].reshape(NCHUNK, 2, B, GCH, DS)
        outs.append(r.transpose(2, 0, 3, 1, 4).reshape(B, NPC, DS))
    full = np.concatenate(outs, axis=1)  # [B, N, DS]
    return np.ascontiguousarray(full).reshape(B, N, D, S)
